# revision 6
# baseline (speedup 1.0000x reference)
"""Trainium2 Bass kernel for nn_DecoderRNN (attention LSTM decoder).

Strategy: pure data-parallel over batch (16 rows/core on 8 cores, no
collectives).  Everything on-device is feature-major ("transposed") so all
elementwise work runs on full 128-partition tiles; all matmuls are
weight-stationary with bf16 operands (fp32 PSUM accumulation, fp32 cell
state).  The per-timestep x-dependent contributions to the attention scores
and the LSTM gates are precomputed on-device for all timesteps in two big
matmuls; each step's scores/gates PSUM banks are initialized by an
identity-stationary matmul (PE-only PSUM writes, no cross-engine bank
hazards).  Scores and gates live in *separate* PSUM banks so the exp() of
the attention scores never serializes against the gate accumulation.  The
vocab projection streams W_out as the moving operand against a stationary
hidden-state chunk, interleaved into the recurrence; the second half of the
x-precompute is interleaved into the early steps' PE bubbles.

sigmoid(x) is computed as 0.5 + 0.5*tanh(x/2) (the 0.5 folded into the
i/f/o weight rows on the host) so the whole kernel needs only the exp/tanh
ACT table set.
"""
import sys
import numpy as np

sys.path.insert(0, "/opt/trn_rl_repo")

import ml_dtypes

B, T, E, H, V, A = 128, 31, 512, 512, 10000, 2048
NCORES = 8
BL = B // NCORES          # 16 batch rows per core
NT = T + 1                # 32 timesteps incl. t=0
R = T * BL                # 496 gathered tokens per core
RPAD = 512                # padded so num_idxs % 128 == 0
G4 = 4 * H                # 2048 gate rows
NX = NT * BL              # 512 hidden cols per core

# vocab projection: chunks of 128 hidden cols x width-1024 vocab groups
NCH = NX // 128           # 4 chunks of (t,b) columns
VGW = 1024
NVG = (V + VGW - 1) // VGW  # 10 groups (last = 784)

BF16 = ml_dtypes.bfloat16
_BUILT = {}


def _bf(x):
    return np.ascontiguousarray(np.asarray(x, np.float32), dtype=BF16)


def _fmajor(x2d):
    """[128*ntile, cols] -> [128, ntile*cols], tile-major feature layout."""
    rows, cols = x2d.shape
    nt = rows // 128
    return np.ascontiguousarray(
        x2d.reshape(nt, 128, cols).transpose(1, 0, 2).reshape(128, nt * cols)
    )


def _build_program():
    """Build the Bass program (single SPMD program, 8 cores)."""
    import concourse.bass as bass
    import concourse.mybir as mybir
    from concourse import tile as tile_mod
    from concourse.tile import TileContext

    def _drain_and_barrier(self, tick_clock, wait_clock):
        drain_inst = self.nc.sync.drain()
        wait_clock.add_sem_waits(
            drain_inst.ins, tile_mod.ScopedClock({None: tick_clock.global_clock})
        )
        self.nc.all_engine_barrier()
        assert self.sems is not None
        popped = self.nc._tile_sem_poison_stack.pop()
        assert popped is self._sem_poison
        self.nc.clear_and_free_semaphores(list(self.sems.allocated().values()))
        self.nc.all_engine_barrier()

    TileContext._drain_and_barrier = _drain_and_barrier

    fp32 = mybir.dt.float32
    bf16 = mybir.dt.bfloat16
    TANH = mybir.ActivationFunctionType.Tanh
    EXP = mybir.ActivationFunctionType.Exp
    MUL = mybir.AluOpType.mult
    ADD = mybir.AluOpType.add

    nc = bass.Bass("TRN2", target_bir_lowering=False)

    # ---- I/O (all weight tensors pre-tiled [128, ntile*cols] on host) ---
    d_xsT = nc.declare_dram_parameter("xsT", [128, 4, RPAD], bf16, isOutput=False)
    d_featT = nc.declare_dram_parameter("featT", [128, 4 * BL], bf16, isOutput=False)
    d_cnnT = nc.declare_dram_parameter("cnnT", [128, 16 * BL], bf16, isOutput=False)
    d_wanhT = nc.declare_dram_parameter("wanhT", [128, 4 * A], bf16, isOutput=False)
    d_wanxT = nc.declare_dram_parameter("wanxT", [128, 4 * A], bf16, isOutput=False)
    d_wadT = nc.declare_dram_parameter("wadT", [128, 16 * E], bf16, isOutput=False)
    d_mxT = nc.declare_dram_parameter("mxT", [128, 4 * G4], bf16, isOutput=False)
    d_wihT = nc.declare_dram_parameter("wihT", [128, 4 * G4], bf16, isOutput=False)
    d_whhT = nc.declare_dram_parameter("whhT", [128, 4 * G4], bf16, isOutput=False)
    d_woutT = nc.declare_dram_parameter("woutT", [128, 4, V], bf16, isOutput=False)
    d_battn = nc.declare_dram_parameter("battn", [1, A], bf16, isOutput=False)
    d_bg = nc.declare_dram_parameter("bg", [1, G4], bf16, isOutput=False)
    d_preg0 = nc.declare_dram_parameter("preg0", [128, 16 * BL], bf16, isOutput=False)
    d_ones = nc.declare_dram_parameter("onesrow", [1, RPAD], bf16, isOutput=False)
    d_onesf = nc.declare_dram_parameter("onesf", [1, 128], fp32, isOutput=False)
    d_onescol = nc.declare_dram_parameter("onescol", [128, 1], bf16, isOutput=False)
    d_onesI = nc.declare_dram_parameter("onesI", [128, 128], bf16, isOutput=False)
    d_zrow = nc.declare_dram_parameter("zrow", [128, 16 * BL], bf16, isOutput=False)
    d_logits = nc.declare_dram_parameter("logits", [NX, V], bf16, isOutput=True)

    with TileContext(nc) as tc:
        with tc.tile_pool(name="persist", bufs=1) as pw, \
             tc.tile_pool(name="psums", bufs=1, space="PSUM") as pp, \
             tc.tile_pool(name="scratch", bufs=2) as sc:
            wanhT = pw.tile([128, 4, A], bf16)
            wadT = pw.tile([128, 16, E], bf16)
            wihT = pw.tile([128, 4, G4], bf16)
            whhT = pw.tile([128, 4, G4], bf16)
            featT = pw.tile([128, 4, BL], bf16)
            cnnT = pw.tile([128, 16, BL], bf16)
            battn = pw.tile([1, A], bf16)
            bg = pw.tile([1, G4], bf16)
            preg0 = pw.tile([128, 16, BL], bf16)
            onesrow = pw.tile([1, RPAD], bf16)
            onesf = pw.tile([1, 128], fp32)
            onescol = pw.tile([128, 1], bf16)
            onesI = pw.tile([128, 128], bf16)
            zrow = pw.tile([128, 16, BL], bf16)
            # x-dependent precompute, rs-block major:
            # [:, rsb, 0:16, :] = attention-score x-part for step rsb+1
            # [:, rsb, 16:32, :] = gate x-part for step rsb+1
            preAG = pw.tile([128, 32, 32, BL], bf16)
            hidT = pw.tile([128, 4, NX], bf16)
            c_sb = pw.tile([128, 4, BL], fp32)

            # step-0 + precompute dependencies first, wout groups last
            for dst, src in [
                (featT[:, :, :], d_featT), (preg0[:, :, :], d_preg0),
                (onesI[:, :], d_onesI), (zrow[:, :, :], d_zrow),
                (onescol[:, :], d_onescol), (onesf[:, :], d_onesf),
                (wihT[:, :, :], d_wihT),
                (battn[:, :], d_battn), (bg[:, :], d_bg),
                (onesrow[:, :], d_ones),
                (wanhT[:, :, :], d_wanhT),
                (whhT[:, :, :], d_whhT), (wadT[:, :, :], d_wadT),
                (cnnT[:, :, :], d_cnnT),
            ]:
                nc.sync.dma_start(out=dst, in_=src[:, :])

            # ---- PSUM layout --------------------------------------------
            # ssc(t) bank: [0:16]=scores, [16,row0]=softmax denom partials,
            #              [20:24]=1/Z broadcast.  sgt(t) bank: [0:16]=gates,
            #              [16:20]=wad output (attended x2 pre-normalize).
            # Every epoch begins with an identity-stationary matmul pair
            # (start=True data copy + start=False zero fill) so every region
            # is TensorE-written before the start=False accumulations.
            def alloc_init(tag, src_ap):
                t_ps = pp.tile([128, 32, BL], fp32, tag=tag, bufs=2)
                nc.tensor.matmul(
                    t_ps[:, 0:16, :], onesI[:, :], src_ap,
                    start=True, stop=False, skip_group_check=True,
                )
                nc.tensor.matmul(
                    t_ps[:, 16:32, :], onesI[:, :], zrow[:, :, :],
                    start=False, stop=True, skip_group_check=True,
                )
                return t_ps

            def alloc_ssc(t):
                return alloc_init("ssc", preAG[:, t - 1, 0:16, :])

            def alloc_sgt(t):
                return alloc_init("sgt", preAG[:, t - 1, 16:32, :])

            def lstm_tail(sgt_ps, t, first):
                tg = sc.tile([128, 16, BL], fp32, tag="tg")
                nc.scalar.activation(tg[:, :, :], sgt_ps[:, 0:16, :], TANH)
                sig = sc.tile([128, 16, BL], fp32, tag="sig")
                nc.vector.tensor_scalar(sig[:, :, :], tg[:, :, :], 0.5, 0.5, MUL, ADD)
                ig = sc.tile([128, 4, BL], fp32, tag="ig")
                nc.vector.tensor_mul(ig[:, :, :], sig[:, 0:4, :], tg[:, 8:12, :])
                if first:
                    nc.vector.tensor_copy(c_sb[:, :, :], ig[:, :, :])
                else:
                    fc = sc.tile([128, 4, BL], fp32, tag="fc")
                    nc.vector.tensor_mul(fc[:, :, :], sig[:, 4:8, :], c_sb[:, :, :])
                    nc.vector.tensor_add(c_sb[:, :, :], ig[:, :, :], fc[:, :, :])
                tc2 = sc.tile([128, 4, BL], fp32, tag="tc2")
                nc.scalar.activation(tc2[:, :, :], c_sb[:, :, :], TANH)
                nc.vector.tensor_mul(
                    hidT[:, :, BL * t:BL * t + BL], sig[:, 12:16, :], tc2[:, :, :]
                )

            evac_ctr = [0]

            def precompute_chunk(ppool, wanxT, mxT, xsT, which, j, half):
                """One (proj, j, rs-half) precompute unit -> preAG."""
                wsrc, brow, joff = (
                    (wanxT, battn, 0) if which == 0 else (mxT, bg, 16)
                )
                rs0 = 256 * half
                ps = pp.tile([128, 1024], fp32, tag="proj", bufs=2)
                for kk in range(4):
                    nc.tensor.matmul(
                        ps[:, 0:256],
                        wsrc[:, kk, 128 * j:128 * j + 128],
                        xsT[:, kk, rs0:rs0 + 256],
                        start=(kk == 0), stop=False,
                    )
                nc.tensor.matmul(
                    ps[:, 0:256],
                    brow[0:1, 128 * j:128 * j + 128],
                    onesrow[0:1, rs0:rs0 + 256],
                    start=False, stop=True,
                )
                e = evac_ctr[0]
                evac_ctr[0] += 1
                dst = preAG[:, 16 * half:16 * half + 16, joff + j, :]
                if e % 2 == 0:
                    nc.vector.tensor_copy(dst, ps[:, 0:256])
                else:
                    nc.scalar.copy(dst, ps[:, 0:256])

            def proj_unit(woutT, ci, g):
                """chunk ci of hid cols, vocab group g: stream wout."""
                c0 = 128 * ci
                v0 = VGW * g
                gw = min(VGW, V - v0)
                ps = pp.tile([128, 1024], fp32, tag="proj", bufs=2)
                for kk in range(4):
                    for hf in range(2):
                        w0 = v0 + 512 * hf
                        m = min(512, v0 + gw - w0)
                        if m <= 0:
                            continue
                        nc.tensor.matmul(
                            ps[:, 512 * hf:512 * hf + m],
                            hidT[:, kk, c0:c0 + 128],
                            woutT[:, kk, w0:w0 + m],
                            start=(kk == 0), stop=(kk == 3),
                        )
                st = sc.tile([128, 1024], bf16, tag="vstage", bufs=4)
                e = evac_ctr[0]
                evac_ctr[0] += 1
                if e % 2 == 0:
                    nc.vector.tensor_copy(st[:, 0:gw], ps[:, 0:gw])
                else:
                    nc.scalar.copy(st[:, 0:gw], ps[:, 0:gw])
                nc.sync.dma_start(
                    out=d_logits[c0:c0 + 128, v0:v0 + gw],
                    in_=st[:, 0:gw],
                )

            def step(t, ssc, sgt):
                rs = BL * (t - 1)

                # attention scores accumulate on top of the preA init
                for j in range(16):
                    for kk in range(4):
                        nc.tensor.matmul(
                            ssc[:, j, :],
                            wanhT[:, kk, 128 * j:128 * j + 128],
                            hidT[:, kk, rs:rs + BL],
                            start=False, stop=(kk == 3),
                            skip_group_check=True,
                        )
                # W_hh @ h into the gate bank; runs under the softmax chain
                for j in range(16):
                    for kk in range(4):
                        nc.tensor.matmul(
                            sgt[:, j, :],
                            whhT[:, kk, 128 * j:128 * j + 128],
                            hidT[:, kk, rs:rs + BL],
                            start=False, stop=False,
                            skip_group_check=True,
                        )

                att = sc.tile([128, 16, BL], bf16, tag="att")
                nc.scalar.activation(att[:, :, :], ssc[:, 0:16, :], EXP)
                att2 = sc.tile([128, 16, BL], bf16, tag="att2")
                nc.vector.tensor_mul(att2[:, :, :], att[:, :, :], cnnT[:, :, :])

                # softmax denominator into ssc bank row (start=False onto the
                # zero fill; accumulates across the 16 j groups)
                ps_z = ssc[0:1, 16, :]
                for j in range(16):
                    nc.tensor.matmul(
                        ps_z,
                        onescol[:, 0:1],
                        att[:, j, :],
                        start=False, stop=(j == 15),
                        skip_group_check=True,
                    )
                rz = sc.tile([1, 4 * BL], fp32, tag="rz")
                nc.vector.reciprocal(rz[0:1, 0:BL], ps_z)
                for q in range(1, 4):
                    nc.vector.tensor_copy(rz[0:1, BL * q:BL * q + BL], rz[0:1, 0:BL])

                # attended-x2 (pre-normalization) into the gate bank's spare
                # groups; the 1/Z broadcast matmul is slotted mid-stream so
                # the PE reaches it just as the reciprocal lands.
                ps_rz = ssc[:, 20:24, :]
                for me in range(4):
                    for ka in range(16):
                        nc.tensor.matmul(
                            sgt[:, 16 + me, :],
                            wadT[:, ka, 128 * me:128 * me + 128],
                            att2[:, ka, :],
                            start=False, stop=(ka == 15),
                            skip_group_check=True,
                        )
                    if me == 2:
                        nc.tensor.matmul(
                            ps_rz, onesf[0:1, :], rz[0:1, :],
                            start=False, stop=True,
                            skip_group_check=True,
                        )
                rzbc = sc.tile([128, 4, BL], fp32, tag="rzbc")
                nc.vector.tensor_copy(rzbc[:, :, :], ps_rz)
                x2aT = sc.tile([128, 4, BL], bf16, tag="x2aT")
                nc.vector.tensor_mul(x2aT[:, :, :], sgt[:, 16:20, :], rzbc[:, :, :])

                for j in range(16):
                    for kk in range(4):
                        nc.tensor.matmul(
                            sgt[:, j, :],
                            wihT[:, kk, 128 * j:128 * j + 128],
                            x2aT[:, kk, :],
                            start=False, stop=(kk == 3),
                            skip_group_check=True,
                        )
                lstm_tail(sgt, t, False)

            # ---- prologue: step 0 + precompute half 1 -------------------
            with tc.tile_pool(name="pre", bufs=1) as ppre:
                wanxT = ppre.tile([128, 4, A], bf16)
                mxT = ppre.tile([128, 4, G4], bf16)
                xsT = ppre.tile([128, 4, RPAD], bf16)
                nc.sync.dma_start(out=xsT[:, :, :], in_=d_xsT[:, :, :])
                nc.sync.dma_start(out=wanxT[:, :, :], in_=d_wanxT[:, :])
                nc.sync.dma_start(out=mxT[:, :, :], in_=d_mxT[:, :])

                # step 0: plain LSTM from zero state
                sg0 = pp.tile([128, 32, BL], fp32, tag="sgt", bufs=2)
                nc.tensor.matmul(
                    sg0[:, 0:16, :], onesI[:, :], preg0[:, :, :],
                    start=True, stop=False, skip_group_check=True,
                )
                nc.tensor.matmul(
                    sg0[:, 16:32, :], onesI[:, :], zrow[:, :, :],
                    start=False, stop=True, skip_group_check=True,
                )
                for j in range(16):
                    for kk in range(4):
                        nc.tensor.matmul(
                            sg0[:, j, :],
                            wihT[:, kk, 128 * j:128 * j + 128],
                            featT[:, kk, :],
                            start=False, stop=(kk == 3),
                            skip_group_check=True,
                        )
                lstm_tail(sg0, 0, True)

                # precompute half 1 (steps 1..16)
                for which in range(2):
                    for j in range(16):
                        precompute_chunk(ppre, wanxT, mxT, xsT, which, j, 0)

                # preloads for steps 1 and 2
                ssc1 = alloc_ssc(1)
                sgt1 = alloc_sgt(1)
                ssc2 = alloc_ssc(2)
                sgt2 = alloc_sgt(2)

                # remaining precompute (half 2) chunks, interleaved below
                pre2 = [(w, j) for w in range(2) for j in range(16)]
                pre2_sched = {}
                for i, u in enumerate(pre2):
                    pre2_sched.setdefault(1 + i * 7 // len(pre2), []).append(u)

                cur = (ssc1, sgt1)
                nxt = (ssc2, sgt2)
                for t in range(1, 8):
                    step(t, cur[0], cur[1])
                    for (w, j) in pre2_sched.get(t, ()):
                        precompute_chunk(ppre, wanxT, mxT, xsT, w, j, 1)
                    nxt2 = (alloc_ssc(t + 2), alloc_sgt(t + 2))
                    cur, nxt = nxt, nxt2

            # wout loads after the pre pool frees its SBUF; per-group DMAs so
            # early projection units only wait on their own group.
            with tc.tile_pool(name="wout", bufs=1) as pwo:
                woutT = pwo.tile([128, 4, V], bf16)
                for g in range(NVG):
                    v0 = VGW * g
                    gw = min(VGW, V - v0)
                    nc.sync.dma_start(
                        out=woutT[:, :, v0:v0 + gw],
                        in_=d_woutT[:, :, v0:v0 + gw],
                    )

                # chunk ci (hidden cols 128*ci..) is final after step 8*ci+7;
                # spread its 10 vocab groups over steps 8*ci+8 .. 8*ci+15.
                proj_sched = {}
                for ci in range(NCH - 1):
                    for s in range(8):
                        t0 = 8 * ci + 8 + s
                        units = range(NVG * s // 8, NVG * (s + 1) // 8)
                        proj_sched.setdefault(t0, []).extend((ci, g) for g in units)

                for t in range(8, NT):
                    step(t, cur[0], cur[1])
                    for (ci, g) in proj_sched.get(t, ()):
                        proj_unit(woutT, ci, g)
                    if t + 2 < NT:
                        nxt2 = (alloc_ssc(t + 2), alloc_sgt(t + 2))
                    else:
                        nxt2 = (None, None)
                    cur, nxt = nxt, nxt2

                for g in range(NVG):
                    proj_unit(woutT, NCH - 1, g)

    # post-pass: walrus in this container allows only 1 sem wait per
    # instruction; move extras onto same-engine NoOps inserted just before.
    nid = 0
    for f in nc.m.functions:
        for bb in f.blocks:
            insts = bb.instructions
            i = 0
            while i < len(insts):
                ins = insts[i]
                si = ins.sync_info
                if si is not None and len(si.on_wait) > 1:
                    waits = list(si.on_wait)
                    si.on_wait = waits[-1:]
                    for w in waits[:-1]:
                        nid += 1
                        nop = mybir.InstNoOp(
                            name=f"WS-{nid}",
                            sync_info=mybir.SyncInfo(on_wait=[w], on_update=[]),
                            bass_nofuse=True,
                            engine=ins.engine,
                        )
                        insts.insert(i, nop)
                        i += 1
                i += 1
    return nc


def _prep_inputs(inputs):
    f32 = np.float32
    features = np.asarray(inputs["features"], f32)
    cnn = np.asarray(inputs["cnn_features"], f32)
    captions = np.asarray(inputs["captions"])
    emb = np.asarray(inputs["embed_table"], f32)
    W_ih = np.asarray(inputs["W_ih"], f32)
    W_hh = np.asarray(inputs["W_hh"], f32)
    b_ih = np.asarray(inputs["b_ih"], f32)
    b_hh = np.asarray(inputs["b_hh"], f32)
    W_attn = np.asarray(inputs["W_attn"], f32)
    b_attn = np.asarray(inputs["b_attn"], f32)
    W_attd = np.asarray(inputs["W_attd"], f32)
    b_attd = np.asarray(inputs["b_attd"], f32)
    W_out = np.asarray(inputs["W_out"], f32)

    s = np.ones((G4, 1), f32)
    s[0:H] = 0.5
    s[H:2 * H] = 0.5
    s[3 * H:4 * H] = 0.5
    Mx = W_ih @ W_attd[:, :E]
    bias_g = (b_ih + b_hh + W_ih @ b_attd) * s[:, 0]
    bias_g0 = (b_ih + b_hh) * s[:, 0]
    preg0 = np.repeat(bias_g0[:, None], BL, axis=1)       # [G4, BL]

    common = {
        "wanhT": _fmajor(_bf(W_attn[:, E:].T)),
        "wanxT": _fmajor(_bf(W_attn[:, :E].T)),
        "wadT": _fmajor(_bf(W_attd[:, E:].T)),
        "mxT": _fmajor(_bf((Mx * s).T)),
        "wihT": _fmajor(_bf((W_ih * s).T)),
        "whhT": _fmajor(_bf((W_hh * s).T)),
        "woutT": _fmajor(_bf(W_out.T)).reshape(128, 4, V),
        "battn": _bf(b_attn[None, :]),
        "bg": _bf(bias_g[None, :]),
        "preg0": _fmajor(_bf(preg0)),
        "onesrow": _bf(np.ones((1, RPAD), f32)),
        "onesf": np.ones((1, 128), f32),
        "onescol": _bf(np.ones((128, 1), f32)),
        "onesI": _bf(np.eye(128, dtype=f32)),
        "zrow": _bf(np.zeros((128, 16 * BL), f32)),
    }
    in_maps = []
    for k in range(NCORES):
        bsl = slice(BL * k, BL * k + BL)
        toks = captions[bsl].astype(np.int64).T.reshape(-1)   # r=(t-1)*16+b
        xs = np.zeros((RPAD, E), np.float32)
        xs[:R] = emb[toks]
        in_maps.append({
            **common,
            "xsT": _fmajor(_bf(xs.T)).reshape(128, 4, RPAD),
            "featT": _fmajor(_bf(features[bsl].T)),
            "cnnT": _fmajor(_bf(cnn[bsl].T)),
        })
    return in_maps


def kernel(**inputs):
    from concourse.bass_utils import run_bass_kernel_spmd

    if "nc" not in _BUILT:
        _BUILT["nc"] = _build_program()
    nc = _BUILT["nc"]
    in_maps = _prep_inputs(inputs)
    res = run_bass_kernel_spmd(nc, in_maps, list(range(NCORES)))

    b_out = np.asarray(inputs["b_out"], np.float32)
    out = np.empty((NT * B, V), np.float32)
    o3 = out.reshape(NT, B, V)
    for k in range(NCORES):
        lt = res.results[k]["logits"]                        # [512, V] bf16
        o3[:, BL * k:BL * k + BL, :] = np.asarray(lt, np.float32).reshape(NT, BL, V)
    out += b_out[None, :]
    return out


# revision 16
# speedup vs baseline: 1.0074x; 1.0074x over previous
"""Trainium2 Bass kernel for nn_DecoderRNN (attention LSTM decoder).

Strategy: pure data-parallel over batch (16 rows/core on 8 cores, no
collectives).  Everything on-device is feature-major ("transposed") so all
elementwise work runs on full 128-partition tiles; all matmuls are
weight-stationary with bf16 operands (fp32 PSUM accumulation, fp32 cell
state).  The per-timestep x-dependent contributions to the attention scores
and the LSTM gates are precomputed on-device for all timesteps in two big
matmuls; each step's scores/gates PSUM banks are initialized by an
identity-stationary matmul (PE-only PSUM writes, no cross-engine bank
hazards).  Scores and gates live in *separate* PSUM banks so the exp() of
the attention scores never serializes against the gate accumulation.  The
vocab projection streams W_out as the moving operand against a stationary
hidden-state chunk, interleaved into the recurrence; the second half of the
x-precompute is interleaved into the early steps' PE bubbles.

sigmoid(x) is computed as 0.5 + 0.5*tanh(x/2) (the 0.5 folded into the
i/f/o weight rows on the host) so the whole kernel needs only the exp/tanh
ACT table set.
"""
import sys
import numpy as np

sys.path.insert(0, "/opt/trn_rl_repo")

import ml_dtypes

B, T, E, H, V, A = 128, 31, 512, 512, 10000, 2048
NCORES = 8
BL = B // NCORES          # 16 batch rows per core
NT = T + 1                # 32 timesteps incl. t=0
R = T * BL                # 496 gathered tokens per core
RPAD = 512                # padded so num_idxs % 128 == 0
G4 = 4 * H                # 2048 gate rows
NX = NT * BL              # 512 hidden cols per core

# vocab projection: chunks of 128 hidden cols x width-1024 vocab groups
NCH = NX // 128           # 4 chunks of (t,b) columns
VGW = 1024
NVG = (V + VGW - 1) // VGW  # 10 groups (last = 784)

BF16 = ml_dtypes.bfloat16
_BUILT = {}


def _bf(x):
    return np.ascontiguousarray(np.asarray(x, np.float32), dtype=BF16)


def _fmajor(x2d):
    """[128*ntile, cols] -> [128, ntile*cols], tile-major feature layout."""
    rows, cols = x2d.shape
    nt = rows // 128
    return np.ascontiguousarray(
        x2d.reshape(nt, 128, cols).transpose(1, 0, 2).reshape(128, nt * cols)
    )


def _build_program():
    """Build the Bass program (single SPMD program, 8 cores)."""
    import concourse.bass as bass
    import concourse.mybir as mybir
    from concourse import tile as tile_mod
    from concourse.tile import TileContext

    def _drain_and_barrier(self, tick_clock, wait_clock):
        drain_inst = self.nc.sync.drain()
        wait_clock.add_sem_waits(
            drain_inst.ins, tile_mod.ScopedClock({None: tick_clock.global_clock})
        )
        self.nc.all_engine_barrier()
        assert self.sems is not None
        popped = self.nc._tile_sem_poison_stack.pop()
        assert popped is self._sem_poison
        self.nc.clear_and_free_semaphores(list(self.sems.allocated().values()))
        self.nc.all_engine_barrier()

    TileContext._drain_and_barrier = _drain_and_barrier

    fp32 = mybir.dt.float32
    bf16 = mybir.dt.bfloat16
    TANH = mybir.ActivationFunctionType.Tanh
    EXP = mybir.ActivationFunctionType.Exp
    MUL = mybir.AluOpType.mult
    ADD = mybir.AluOpType.add

    nc = bass.Bass("TRN2", target_bir_lowering=False)

    # ---- I/O (all weight tensors pre-tiled [128, ntile*cols] on host) ---
    d_xsT = nc.declare_dram_parameter("xsT", [128, 4, RPAD], bf16, isOutput=False)
    d_featT = nc.declare_dram_parameter("featT", [128, 4 * BL], bf16, isOutput=False)
    d_cnnT = nc.declare_dram_parameter("cnnT", [128, 16 * BL], bf16, isOutput=False)
    d_wanhT = nc.declare_dram_parameter("wanhT", [128, 4 * A], bf16, isOutput=False)
    d_wanxT = nc.declare_dram_parameter("wanxT", [128, 4 * A], bf16, isOutput=False)
    d_wadT = nc.declare_dram_parameter("wadT", [128, 16 * E], bf16, isOutput=False)
    d_mxT = nc.declare_dram_parameter("mxT", [128, 4 * G4], bf16, isOutput=False)
    d_wihT = nc.declare_dram_parameter("wihT", [128, 4 * G4], bf16, isOutput=False)
    d_whhT = nc.declare_dram_parameter("whhT", [128, 4 * G4], bf16, isOutput=False)
    d_woutT = nc.declare_dram_parameter("woutT", [128, 4, V], bf16, isOutput=False)
    d_battn = nc.declare_dram_parameter("battn", [1, A], bf16, isOutput=False)
    d_bg = nc.declare_dram_parameter("bg", [1, G4], bf16, isOutput=False)
    d_preg0 = nc.declare_dram_parameter("preg0", [128, 16 * BL], bf16, isOutput=False)
    d_ones = nc.declare_dram_parameter("onesrow", [1, RPAD], bf16, isOutput=False)
    d_onesf = nc.declare_dram_parameter("onesf", [1, 128], fp32, isOutput=False)
    d_onescol = nc.declare_dram_parameter("onescol", [128, 1], bf16, isOutput=False)
    d_onesI = nc.declare_dram_parameter("onesI", [128, 128], bf16, isOutput=False)
    d_zrow = nc.declare_dram_parameter("zrow", [128, 16 * BL], bf16, isOutput=False)
    d_logits = nc.declare_dram_parameter("logits", [NX, V], bf16, isOutput=True)

    with TileContext(nc) as tc:
        with tc.tile_pool(name="persist", bufs=1) as pw, \
             tc.tile_pool(name="psums", bufs=1, space="PSUM") as pp, \
             tc.tile_pool(name="scratch", bufs=2) as sc:
            wanhT = pw.tile([128, 4, A], bf16)
            wadT = pw.tile([128, 16, E], bf16)
            wihT = pw.tile([128, 4, G4], bf16)
            whhT = pw.tile([128, 4, G4], bf16)
            featT = pw.tile([128, 4, BL], bf16)
            cnnT = pw.tile([128, 16, BL], bf16)
            battn = pw.tile([1, A], bf16)
            bg = pw.tile([1, G4], bf16)
            preg0 = pw.tile([128, 16, BL], bf16)
            onesrow = pw.tile([1, RPAD], bf16)
            onesf = pw.tile([1, 128], fp32)
            onescol = pw.tile([128, 1], bf16)
            onesI = pw.tile([128, 128], bf16)
            zrow = pw.tile([128, 16, BL], bf16)
            # x-dependent precompute, rs-block major:
            # [:, rsb, 0:16, :] = attention-score x-part for step rsb+1
            # [:, rsb, 16:32, :] = gate x-part for step rsb+1
            preAG = pw.tile([128, 32, 32, BL], bf16)
            hidT = pw.tile([128, 4, NX], bf16)
            c_sb = pw.tile([128, 4, BL], fp32)

            # step-0 + precompute dependencies first, wout groups last
            for dst, src in [
                (featT[:, :, :], d_featT), (preg0[:, :, :], d_preg0),
                (onesI[:, :], d_onesI), (zrow[:, :, :], d_zrow),
                (onescol[:, :], d_onescol), (onesf[:, :], d_onesf),
                (battn[:, :], d_battn), (bg[:, :], d_bg),
                (onesrow[:, :], d_ones),
                (wihT[:, :, :], d_wihT),
            ]:
                nc.sync.dma_start(out=dst, in_=src[:, :])

            # ---- PSUM layout --------------------------------------------
            # ssc(t) bank: [0:16]=scores, [16,row0]=softmax denom partials,
            #              [20:24]=1/Z broadcast.  sgt(t) bank: [0:16]=gates,
            #              [16:20]=wad output (attended x2 pre-normalize).
            # Every epoch begins with an identity-stationary matmul pair
            # (start=True data copy + start=False zero fill) so every region
            # is TensorE-written before the start=False accumulations.
            def alloc_init(tag, src_ap, zgrp):
                t_ps = pp.tile([128, 32, BL], fp32, tag=tag, bufs=2)
                nc.tensor.matmul(
                    t_ps[:, 0:16, :], onesI[:, :], src_ap,
                    start=True, stop=False, skip_group_check=True,
                )
                nc.tensor.matmul(
                    t_ps[:, 16:16 + zgrp, :], onesI[:, :], zrow[:, 0:zgrp, :],
                    start=False, stop=True, skip_group_check=True,
                )
                return t_ps

            def alloc_ssc(t):
                # zero groups 16:24 (denom partials @16, 1/Z broadcast @20:24)
                return alloc_init("ssc", preAG[:, t - 1, 0:16, :], 8)

            def alloc_sgt(t):
                # zero groups 16:20 (wad output)
                return alloc_init("sgt", preAG[:, t - 1, 16:32, :], 4)

            def lstm_tail(sgt_ps, t, first):
                tg = sc.tile([128, 16, BL], fp32, tag="tg")
                nc.scalar.activation(tg[:, :, :], sgt_ps[:, 0:16, :], TANH)
                sig = sc.tile([128, 16, BL], fp32, tag="sig")
                nc.vector.tensor_scalar(sig[:, :, :], tg[:, :, :], 0.5, 0.5, MUL, ADD)
                ig = sc.tile([128, 4, BL], fp32, tag="ig")
                nc.vector.tensor_mul(ig[:, :, :], sig[:, 0:4, :], tg[:, 8:12, :])
                if first:
                    nc.vector.tensor_copy(c_sb[:, :, :], ig[:, :, :])
                else:
                    fc = sc.tile([128, 4, BL], fp32, tag="fc")
                    nc.vector.tensor_mul(fc[:, :, :], sig[:, 4:8, :], c_sb[:, :, :])
                    nc.vector.tensor_add(c_sb[:, :, :], ig[:, :, :], fc[:, :, :])
                tc2 = sc.tile([128, 4, BL], fp32, tag="tc2")
                nc.scalar.activation(tc2[:, :, :], c_sb[:, :, :], TANH)
                nc.vector.tensor_mul(
                    hidT[:, :, BL * t:BL * t + BL], sig[:, 12:16, :], tc2[:, :, :]
                )

            evac_ctr = [0]

            def precompute_chunk(ppool, wanxT, mxT, xsT, which, j, half):
                """One (proj, j, rs-half) precompute unit -> preAG."""
                wsrc, brow, joff = (
                    (wanxT, battn, 0) if which == 0 else (mxT, bg, 16)
                )
                rs0 = 256 * half
                ps = pp.tile([128, 1024], fp32, tag="proj", bufs=2)
                for kk in range(4):
                    nc.tensor.matmul(
                        ps[:, 0:256],
                        wsrc[:, kk, 128 * j:128 * j + 128],
                        xsT[:, kk, rs0:rs0 + 256],
                        start=(kk == 0), stop=False,
                    )
                nc.tensor.matmul(
                    ps[:, 0:256],
                    brow[0:1, 128 * j:128 * j + 128],
                    onesrow[0:1, rs0:rs0 + 256],
                    start=False, stop=True,
                )
                e = evac_ctr[0]
                evac_ctr[0] += 1
                dst = preAG[:, 16 * half:16 * half + 16, joff + j, :]
                if e % 2 == 0:
                    nc.vector.tensor_copy(dst, ps[:, 0:256])
                else:
                    nc.scalar.copy(dst, ps[:, 0:256])

            def _proj_half(woutT, ci, g, ps, kks):
                c0 = 128 * ci
                v0 = VGW * g
                gw = min(VGW, V - v0)
                for kk in kks:
                    for hf in range(2):
                        w0 = v0 + 512 * hf
                        m = min(512, v0 + gw - w0)
                        if m <= 0:
                            continue
                        nc.tensor.matmul(
                            ps[:, 512 * hf:512 * hf + m],
                            hidT[:, kk, c0:c0 + 128],
                            woutT[:, kk, w0:w0 + m],
                            start=(kk == 0), stop=(kk == 3),
                        )

            def _proj_evac(ci, g, ps):
                c0 = 128 * ci
                v0 = VGW * g
                gw = min(VGW, V - v0)
                st = sc.tile([128, 1024], bf16, tag="vstage", bufs=4)
                e = evac_ctr[0]
                evac_ctr[0] += 1
                if e % 2 == 0:
                    nc.vector.tensor_copy(st[:, 0:gw], ps[:, 0:gw])
                else:
                    nc.scalar.copy(st[:, 0:gw], ps[:, 0:gw])
                nc.sync.dma_start(
                    out=d_logits[c0:c0 + 128, v0:v0 + gw],
                    in_=st[:, 0:gw],
                )

            def proj_pieces(woutT, ci, g):
                """Two ~half-size filler pieces for one projection unit."""
                state = {}

                def piece_a():
                    ps = pp.tile([128, 1024], fp32, tag="proj", bufs=2)
                    state["ps"] = ps
                    _proj_half(woutT, ci, g, ps, (0, 1))

                def piece_b():
                    ps = state["ps"]
                    _proj_half(woutT, ci, g, ps, (2, 3))
                    _proj_evac(ci, g, ps)

                return [piece_a, piece_b]

            def proj_unit(woutT, ci, g):
                a, b = proj_pieces(woutT, ci, g)
                a()
                b()

            def step(t, ssc, sgt, fill_a=(), fill_b=()):
                rs = BL * (t - 1)

                # attention scores accumulate on top of the preA init
                for j in range(16):
                    for kk in range(4):
                        nc.tensor.matmul(
                            ssc[:, j, :],
                            wanhT[:, kk, 128 * j:128 * j + 128],
                            hidT[:, kk, rs:rs + BL],
                            start=False, stop=(kk == 3),
                            skip_group_check=True,
                        )
                # W_hh @ h into the gate bank; runs under the softmax chain
                for j in range(16):
                    for kk in range(4):
                        nc.tensor.matmul(
                            sgt[:, j, :],
                            whhT[:, kk, 128 * j:128 * j + 128],
                            hidT[:, kk, rs:rs + BL],
                            start=False, stop=False,
                            skip_group_check=True,
                        )

                att = sc.tile([128, 16, BL], bf16, tag="att")
                nc.scalar.activation(att[:, :, :], ssc[:, 0:16, :], EXP)
                att2 = sc.tile([128, 16, BL], bf16, tag="att2")
                nc.vector.tensor_mul(att2[:, :, :], att[:, :, :], cnnT[:, :, :])

                # filler: keeps the PE busy while exp/att2 land
                for f in fill_a:
                    f()

                # softmax denominator into ssc bank row (start=False onto the
                # zero fill; accumulates across the 16 j groups)
                ps_z = ssc[0:1, 16, :]
                for j in range(16):
                    nc.tensor.matmul(
                        ps_z,
                        onescol[:, 0:1],
                        att[:, j, :],
                        start=False, stop=(j == 15),
                        skip_group_check=True,
                    )
                rz = sc.tile([1, 4 * BL], fp32, tag="rz")
                nc.vector.reciprocal(rz[0:1, 0:BL], ps_z)
                for q in range(1, 4):
                    nc.vector.tensor_copy(rz[0:1, BL * q:BL * q + BL], rz[0:1, 0:BL])

                # attended-x2 (pre-normalization) into the gate bank's spare
                # groups; the 1/Z broadcast matmul is slotted mid-stream so
                # the PE reaches it just as the reciprocal lands.
                ps_rz = ssc[:, 20:24, :]
                for me in range(4):
                    for ka in range(16):
                        nc.tensor.matmul(
                            sgt[:, 16 + me, :],
                            wadT[:, ka, 128 * me:128 * me + 128],
                            att2[:, ka, :],
                            start=False, stop=(ka == 15),
                            skip_group_check=True,
                        )
                    if me == 2:
                        nc.tensor.matmul(
                            ps_rz, onesf[0:1, :], rz[0:1, :],
                            start=False, stop=True,
                            skip_group_check=True,
                        )
                rzbc = sc.tile([128, 4, BL], fp32, tag="rzbc")
                nc.vector.tensor_copy(rzbc[:, :, :], ps_rz)
                x2aT = sc.tile([128, 4, BL], bf16, tag="x2aT")
                nc.vector.tensor_mul(x2aT[:, :, :], sgt[:, 16:20, :], rzbc[:, :, :])

                # filler: keeps the PE busy while rzbc/x2aT land
                for f in fill_b:
                    f()

                for j in range(16):
                    for kk in range(4):
                        nc.tensor.matmul(
                            sgt[:, j, :],
                            wihT[:, kk, 128 * j:128 * j + 128],
                            x2aT[:, kk, :],
                            start=False, stop=(kk == 3),
                            skip_group_check=True,
                        )
                lstm_tail(sgt, t, False)

            # ---- prologue: step 0 + precompute half 1 -------------------
            with tc.tile_pool(name="pre", bufs=1) as ppre:
                wanxT = ppre.tile([128, 4, A], bf16)
                mxT = ppre.tile([128, 4, G4], bf16)
                xsT = ppre.tile([128, 4, RPAD], bf16)
                nc.sync.dma_start(out=xsT[:, :, :], in_=d_xsT[:, :, :])
                nc.sync.dma_start(out=wanxT[:, :, :], in_=d_wanxT[:, :])
                nc.sync.dma_start(out=mxT[:, :, :], in_=d_mxT[:, :])
                # step-1+ weights follow the precompute inputs on the queue
                nc.sync.dma_start(out=wanhT[:, :, :], in_=d_wanhT[:, :])
                nc.sync.dma_start(out=whhT[:, :, :], in_=d_whhT[:, :])
                nc.sync.dma_start(out=wadT[:, :, :], in_=d_wadT[:, :])
                nc.sync.dma_start(out=cnnT[:, :, :], in_=d_cnnT[:, :])

                # step 0: plain LSTM from zero state
                sg0 = pp.tile([128, 32, BL], fp32, tag="sgt", bufs=2)
                nc.tensor.matmul(
                    sg0[:, 0:16, :], onesI[:, :], preg0[:, :, :],
                    start=True, stop=False, skip_group_check=True,
                )
                nc.tensor.matmul(
                    sg0[:, 16:32, :], onesI[:, :], zrow[:, :, :],
                    start=False, stop=True, skip_group_check=True,
                )
                for j in range(16):
                    for kk in range(4):
                        nc.tensor.matmul(
                            sg0[:, j, :],
                            wihT[:, kk, 128 * j:128 * j + 128],
                            featT[:, kk, :],
                            start=False, stop=(kk == 3),
                            skip_group_check=True,
                        )
                lstm_tail(sg0, 0, True)

                # precompute half 1 (steps 1..16)
                for which in range(2):
                    for j in range(16):
                        precompute_chunk(ppre, wanxT, mxT, xsT, which, j, 0)

                # preloads for steps 1 and 2
                ssc1 = alloc_ssc(1)
                sgt1 = alloc_sgt(1)
                ssc2 = alloc_ssc(2)
                sgt2 = alloc_sgt(2)

                # remaining precompute (half 2) chunks, interleaved below
                pre2 = [(w, j) for w in range(2) for j in range(16)]
                pre2_sched = {}
                for i, u in enumerate(pre2):
                    pre2_sched.setdefault(1 + i * 7 // len(pre2), []).append(u)

                cur = (ssc1, sgt1)
                nxt = (ssc2, sgt2)
                for t in range(1, 8):
                    chunks = [
                        (lambda w=w, j=j:
                         precompute_chunk(ppre, wanxT, mxT, xsT, w, j, 1))
                        for (w, j) in pre2_sched.get(t, ())
                    ]
                    step(t, cur[0], cur[1],
                         fill_a=chunks[0:1], fill_b=chunks[1:2])
                    for f in chunks[2:]:
                        f()
                    nxt2 = (alloc_ssc(t + 2), alloc_sgt(t + 2))
                    cur, nxt = nxt, nxt2

            # wout loads after the pre pool frees its SBUF; per-group DMAs so
            # early projection units only wait on their own group.
            with tc.tile_pool(name="wout", bufs=1) as pwo:
                woutT = pwo.tile([128, 4, V], bf16)
                for g in range(NVG):
                    v0 = VGW * g
                    gw = min(VGW, V - v0)
                    nc.sync.dma_start(
                        out=woutT[:, :, v0:v0 + gw],
                        in_=d_woutT[:, :, v0:v0 + gw],
                    )

                # chunk ci (hidden cols 128*ci..) is final after step 8*ci+7;
                # spread its 10 vocab groups over steps 8*ci+8 .. 8*ci+15.
                proj_sched = {}
                for ci in range(NCH - 1):
                    for s in range(8):
                        t0 = 8 * ci + 8 + s
                        units = range(NVG * s // 8, NVG * (s + 1) // 8)
                        proj_sched.setdefault(t0, []).extend((ci, g) for g in units)

                for t in range(8, NT):
                    pieces = []
                    for (ci, g) in proj_sched.get(t, ()):
                        pieces.extend(proj_pieces(woutT, ci, g))
                    step(t, cur[0], cur[1],
                         fill_a=pieces[0:1], fill_b=pieces[1:2])
                    for f in pieces[2:]:
                        f()
                    if t + 2 < NT:
                        nxt2 = (alloc_ssc(t + 2), alloc_sgt(t + 2))
                    else:
                        nxt2 = (None, None)
                    cur, nxt = nxt, nxt2

                for g in range(NVG):
                    proj_unit(woutT, NCH - 1, g)

    # post-pass: walrus in this container allows only 1 sem wait per
    # instruction; move extras onto same-engine NoOps inserted just before.
    nid = 0
    for f in nc.m.functions:
        for bb in f.blocks:
            insts = bb.instructions
            i = 0
            while i < len(insts):
                ins = insts[i]
                si = ins.sync_info
                if si is not None and len(si.on_wait) > 1:
                    waits = list(si.on_wait)
                    si.on_wait = waits[-1:]
                    for w in waits[:-1]:
                        nid += 1
                        nop = mybir.InstNoOp(
                            name=f"WS-{nid}",
                            sync_info=mybir.SyncInfo(on_wait=[w], on_update=[]),
                            bass_nofuse=True,
                            engine=ins.engine,
                        )
                        insts.insert(i, nop)
                        i += 1
                i += 1
    return nc


def _prep_inputs(inputs):
    f32 = np.float32
    features = np.asarray(inputs["features"], f32)
    cnn = np.asarray(inputs["cnn_features"], f32)
    captions = np.asarray(inputs["captions"])
    emb = np.asarray(inputs["embed_table"], f32)
    W_ih = np.asarray(inputs["W_ih"], f32)
    W_hh = np.asarray(inputs["W_hh"], f32)
    b_ih = np.asarray(inputs["b_ih"], f32)
    b_hh = np.asarray(inputs["b_hh"], f32)
    W_attn = np.asarray(inputs["W_attn"], f32)
    b_attn = np.asarray(inputs["b_attn"], f32)
    W_attd = np.asarray(inputs["W_attd"], f32)
    b_attd = np.asarray(inputs["b_attd"], f32)
    W_out = np.asarray(inputs["W_out"], f32)

    s = np.ones((G4, 1), f32)
    s[0:H] = 0.5
    s[H:2 * H] = 0.5
    s[3 * H:4 * H] = 0.5
    Mx = W_ih @ W_attd[:, :E]
    bias_g = (b_ih + b_hh + W_ih @ b_attd) * s[:, 0]
    bias_g0 = (b_ih + b_hh) * s[:, 0]
    preg0 = np.repeat(bias_g0[:, None], BL, axis=1)       # [G4, BL]

    common = {
        "wanhT": _fmajor(_bf(W_attn[:, E:].T)),
        "wanxT": _fmajor(_bf(W_attn[:, :E].T)),
        "wadT": _fmajor(_bf(W_attd[:, E:].T)),
        "mxT": _fmajor(_bf((Mx * s).T)),
        "wihT": _fmajor(_bf((W_ih * s).T)),
        "whhT": _fmajor(_bf((W_hh * s).T)),
        "woutT": _fmajor(_bf(W_out.T)).reshape(128, 4, V),
        "battn": _bf(b_attn[None, :]),
        "bg": _bf(bias_g[None, :]),
        "preg0": _fmajor(_bf(preg0)),
        "onesrow": _bf(np.ones((1, RPAD), f32)),
        "onesf": np.ones((1, 128), f32),
        "onescol": _bf(np.ones((128, 1), f32)),
        "onesI": _bf(np.eye(128, dtype=f32)),
        "zrow": _bf(np.zeros((128, 16 * BL), f32)),
    }
    in_maps = []
    for k in range(NCORES):
        bsl = slice(BL * k, BL * k + BL)
        toks = captions[bsl].astype(np.int64).T.reshape(-1)   # r=(t-1)*16+b
        xs = np.zeros((RPAD, E), np.float32)
        xs[:R] = emb[toks]
        in_maps.append({
            **common,
            "xsT": _fmajor(_bf(xs.T)).reshape(128, 4, RPAD),
            "featT": _fmajor(_bf(features[bsl].T)),
            "cnnT": _fmajor(_bf(cnn[bsl].T)),
        })
    return in_maps


def kernel(**inputs):
    from concourse.bass_utils import run_bass_kernel_spmd

    if "nc" not in _BUILT:
        _BUILT["nc"] = _build_program()
    nc = _BUILT["nc"]
    in_maps = _prep_inputs(inputs)
    res = run_bass_kernel_spmd(nc, in_maps, list(range(NCORES)))

    b_out = np.asarray(inputs["b_out"], np.float32)
    out = np.empty((NT * B, V), np.float32)
    o3 = out.reshape(NT, B, V)
    for k in range(NCORES):
        lt = res.results[k]["logits"]                        # [512, V] bf16
        o3[:, BL * k:BL * k + BL, :] = np.asarray(lt, np.float32).reshape(NT, BL, V)
    out += b_out[None, :]
    return out


# revision 22
# speedup vs baseline: 1.0381x; 1.0304x over previous
"""Trainium2 Bass kernel for nn_DecoderRNN (attention LSTM decoder).

Strategy: pure data-parallel over batch (16 rows/core on 8 cores, no
collectives).  Everything on-device is feature-major ("transposed") so all
elementwise work runs on full 128-partition tiles; all matmuls are
weight-stationary with bf16 operands (fp32 PSUM accumulation, fp32 cell
state).  The per-timestep x-dependent contributions to the attention scores
and the LSTM gates are precomputed on-device for all timesteps in two big
matmuls; each step's scores/gates PSUM banks are initialized by an
identity-stationary matmul (PE-only PSUM writes, no cross-engine bank
hazards).  Scores and gates live in *separate* PSUM banks so the exp() of
the attention scores never serializes against the gate accumulation.  The
vocab projection streams W_out as the moving operand against a stationary
hidden-state chunk, interleaved into the recurrence; the second half of the
x-precompute is interleaved into the early steps' PE bubbles.

sigmoid(x) is computed as 0.5 + 0.5*tanh(x/2) (the 0.5 folded into the
i/f/o weight rows on the host) so the whole kernel needs only the exp/tanh
ACT table set.
"""
import sys
import numpy as np

sys.path.insert(0, "/opt/trn_rl_repo")

import ml_dtypes

B, T, E, H, V, A = 128, 31, 512, 512, 10000, 2048
NCORES = 8
BL = B // NCORES          # 16 batch rows per core
NT = T + 1                # 32 timesteps incl. t=0
R = T * BL                # 496 gathered tokens per core
RPAD = 512                # padded so num_idxs % 128 == 0
G4 = 4 * H                # 2048 gate rows
NX = NT * BL              # 512 hidden cols per core

# vocab projection: chunks of 128 hidden cols x width-1024 vocab groups
NCH = NX // 128           # 4 chunks of (t,b) columns
VGW = 1024
NVG = (V + VGW - 1) // VGW  # 10 groups (last = 784)

BF16 = ml_dtypes.bfloat16
_BUILT = {}


def _bf(x):
    return np.ascontiguousarray(np.asarray(x, np.float32), dtype=BF16)


def _f8(x):
    return np.ascontiguousarray(
        np.asarray(x, np.float32), dtype=ml_dtypes.float8_e4m3fn
    )


def _fmajor(x2d):
    """[128*ntile, cols] -> [128, ntile*cols], tile-major feature layout."""
    rows, cols = x2d.shape
    nt = rows // 128
    return np.ascontiguousarray(
        x2d.reshape(nt, 128, cols).transpose(1, 0, 2).reshape(128, nt * cols)
    )


def _build_program():
    """Build the Bass program (single SPMD program, 8 cores)."""
    import concourse.bass as bass
    import concourse.mybir as mybir
    from concourse import tile as tile_mod
    from concourse.tile import TileContext

    def _drain_and_barrier(self, tick_clock, wait_clock):
        drain_inst = self.nc.sync.drain()
        wait_clock.add_sem_waits(
            drain_inst.ins, tile_mod.ScopedClock({None: tick_clock.global_clock})
        )
        self.nc.all_engine_barrier()
        assert self.sems is not None
        popped = self.nc._tile_sem_poison_stack.pop()
        assert popped is self._sem_poison
        self.nc.clear_and_free_semaphores(list(self.sems.allocated().values()))
        self.nc.all_engine_barrier()

    TileContext._drain_and_barrier = _drain_and_barrier

    fp32 = mybir.dt.float32
    bf16 = mybir.dt.bfloat16
    fp8 = mybir.dt.float8e4
    TANH = mybir.ActivationFunctionType.Tanh
    EXP = mybir.ActivationFunctionType.Exp
    MUL = mybir.AluOpType.mult
    ADD = mybir.AluOpType.add

    nc = bass.Bass("TRN2", target_bir_lowering=False)

    # ---- I/O (all weight tensors pre-tiled [128, ntile*cols] on host) ---
    d_xsT = nc.declare_dram_parameter("xsT", [128, 4, RPAD], bf16, isOutput=False)
    d_featT = nc.declare_dram_parameter("featT", [128, 4 * BL], bf16, isOutput=False)
    d_cnnT = nc.declare_dram_parameter("cnnT", [128, 16 * BL], bf16, isOutput=False)
    d_wanhT = nc.declare_dram_parameter("wanhT", [128, 4 * A], fp8, isOutput=False)
    d_wanxT = nc.declare_dram_parameter("wanxT", [128, 4 * A], bf16, isOutput=False)
    d_wadT = nc.declare_dram_parameter("wadT", [128, 16 * E], fp8, isOutput=False)
    d_mxT = nc.declare_dram_parameter("mxT", [128, 4 * G4], bf16, isOutput=False)
    d_wihT = nc.declare_dram_parameter("wihT", [128, 4 * G4], bf16, isOutput=False)
    d_whhT = nc.declare_dram_parameter("whhT", [128, 4 * G4], bf16, isOutput=False)
    d_woutT = nc.declare_dram_parameter("woutT", [128, 4, V], bf16, isOutput=False)
    d_battn = nc.declare_dram_parameter("battn", [1, A], bf16, isOutput=False)
    d_bg = nc.declare_dram_parameter("bg", [1, G4], bf16, isOutput=False)
    d_preg0 = nc.declare_dram_parameter("preg0", [128, 16 * BL], bf16, isOutput=False)
    d_ones = nc.declare_dram_parameter("onesrow", [1, RPAD], bf16, isOutput=False)
    d_onesf = nc.declare_dram_parameter("onesf", [1, 128], fp32, isOutput=False)
    d_onescol = nc.declare_dram_parameter("onescol", [128, 1], bf16, isOutput=False)
    d_onesI = nc.declare_dram_parameter("onesI", [128, 128], bf16, isOutput=False)
    d_zrow = nc.declare_dram_parameter("zrow", [128, 16 * BL], bf16, isOutput=False)
    d_logits = nc.declare_dram_parameter("logits", [NX, V], bf16, isOutput=True)

    with TileContext(nc) as tc:
        with tc.tile_pool(name="persist", bufs=1) as pw, \
             tc.tile_pool(name="psums", bufs=1, space="PSUM") as pp, \
             tc.tile_pool(name="scratch", bufs=2) as sc:
            wanhT = pw.tile([128, 4, A], fp8)
            wadT = pw.tile([128, 16, E], fp8)
            wihT = pw.tile([128, 4, G4], bf16)
            whhT = pw.tile([128, 4, G4], bf16)
            featT = pw.tile([128, 4, BL], bf16)
            cnnT = pw.tile([128, 16, BL], bf16)
            battn = pw.tile([1, A], bf16)
            bg = pw.tile([1, G4], bf16)
            preg0 = pw.tile([128, 16, BL], bf16)
            onesrow = pw.tile([1, RPAD], bf16)
            onesf = pw.tile([1, 128], fp32)
            onescol = pw.tile([128, 1], bf16)
            onesI = pw.tile([128, 128], bf16)
            zrow = pw.tile([128, 16, BL], bf16)
            # x-dependent precompute, rs-block major:
            # [:, rsb, 0:16, :] = attention-score x-part for step rsb+1
            # [:, rsb, 16:32, :] = gate x-part for step rsb+1
            preAG = pw.tile([128, 32, 32, BL], bf16)
            hidT = pw.tile([128, 4, NX], bf16)
            c_sb = pw.tile([128, 4, BL], fp32)

            # step-0 + precompute dependencies first, wout groups last
            for dst, src in [
                (featT[:, :, :], d_featT), (preg0[:, :, :], d_preg0),
                (onesI[:, :], d_onesI), (zrow[:, :, :], d_zrow),
                (onescol[:, :], d_onescol), (onesf[:, :], d_onesf),
                (battn[:, :], d_battn), (bg[:, :], d_bg),
                (onesrow[:, :], d_ones),
                (wihT[:, :, :], d_wihT),
            ]:
                nc.sync.dma_start(out=dst, in_=src[:, :])

            # ---- PSUM layout --------------------------------------------
            # ssc(t) bank: [0:16]=scores, [16,row0]=softmax denom partials,
            #              [20:24]=1/Z broadcast.  sgt(t) bank: [0:16]=gates,
            #              [16:20]=wad output (attended x2 pre-normalize).
            # Every epoch begins with an identity-stationary matmul pair
            # (start=True data copy + start=False zero fill) so every region
            # is TensorE-written before the start=False accumulations.
            def alloc_init(tag, src_ap, zgrp):
                t_ps = pp.tile([128, 32, BL], fp32, tag=tag, bufs=2)
                nc.tensor.matmul(
                    t_ps[:, 0:16, :], onesI[:, :], src_ap,
                    start=True, stop=False, skip_group_check=True,
                )
                nc.tensor.matmul(
                    t_ps[:, 16:16 + zgrp, :], onesI[:, :], zrow[:, 0:zgrp, :],
                    start=False, stop=True, skip_group_check=True,
                )
                return t_ps

            def alloc_ssc(t):
                # zero groups 16:24 (denom partials @16, 1/Z broadcast @20:24)
                return alloc_init("ssc", preAG[:, t - 1, 0:16, :], 8)

            def alloc_sgt(t):
                # zero groups 16:20 (wad output)
                return alloc_init("sgt", preAG[:, t - 1, 16:32, :], 4)

            def lstm_tail(sgt_ps, t, first):
                tg = sc.tile([128, 16, BL], fp32, tag="tg")
                nc.scalar.activation(tg[:, :, :], sgt_ps[:, 0:16, :], TANH)
                sig = sc.tile([128, 16, BL], fp32, tag="sig")
                nc.vector.tensor_scalar(sig[:, :, :], tg[:, :, :], 0.5, 0.5, MUL, ADD)
                ig = sc.tile([128, 4, BL], fp32, tag="ig")
                nc.vector.tensor_mul(ig[:, :, :], sig[:, 0:4, :], tg[:, 8:12, :])
                if first:
                    nc.vector.tensor_copy(c_sb[:, :, :], ig[:, :, :])
                else:
                    fc = sc.tile([128, 4, BL], fp32, tag="fc")
                    nc.vector.tensor_mul(fc[:, :, :], sig[:, 4:8, :], c_sb[:, :, :])
                    nc.vector.tensor_add(c_sb[:, :, :], ig[:, :, :], fc[:, :, :])
                tc2 = sc.tile([128, 4, BL], fp32, tag="tc2")
                nc.scalar.activation(tc2[:, :, :], c_sb[:, :, :], TANH)
                nc.vector.tensor_mul(
                    hidT[:, :, BL * t:BL * t + BL], sig[:, 12:16, :], tc2[:, :, :]
                )

            evac_ctr = [0]

            def precompute_chunk(ppool, wanxT, mxT, xsT, which, j, half):
                """One (proj, j, rs-half) precompute unit -> preAG."""
                wsrc, brow, joff = (
                    (wanxT, battn, 0) if which == 0 else (mxT, bg, 16)
                )
                rs0 = 256 * half
                ps = pp.tile([128, 1024], fp32, tag="proj", bufs=2)
                for kk in range(4):
                    nc.tensor.matmul(
                        ps[:, 0:256],
                        wsrc[:, kk, 128 * j:128 * j + 128],
                        xsT[:, kk, rs0:rs0 + 256],
                        start=(kk == 0), stop=False,
                    )
                nc.tensor.matmul(
                    ps[:, 0:256],
                    brow[0:1, 128 * j:128 * j + 128],
                    onesrow[0:1, rs0:rs0 + 256],
                    start=False, stop=True,
                )
                e = evac_ctr[0]
                evac_ctr[0] += 1
                dst = preAG[:, 16 * half:16 * half + 16, joff + j, :]
                if e % 2 == 0:
                    nc.vector.tensor_copy(dst, ps[:, 0:256])
                else:
                    nc.scalar.copy(dst, ps[:, 0:256])

            def _proj_half(woutT, ci, g, ps, kks):
                c0 = 128 * ci
                v0 = VGW * g
                gw = min(VGW, V - v0)
                for kk in kks:
                    for hf in range(2):
                        w0 = v0 + 512 * hf
                        m = min(512, v0 + gw - w0)
                        if m <= 0:
                            continue
                        nc.tensor.matmul(
                            ps[:, 512 * hf:512 * hf + m],
                            hidT[:, kk, c0:c0 + 128],
                            woutT[:, kk, w0:w0 + m],
                            start=(kk == 0), stop=(kk == 3),
                        )

            def _proj_evac(ci, g, ps):
                c0 = 128 * ci
                v0 = VGW * g
                gw = min(VGW, V - v0)
                st = sc.tile([128, 1024], bf16, tag="vstage", bufs=4)
                e = evac_ctr[0]
                evac_ctr[0] += 1
                if e % 2 == 0:
                    nc.vector.tensor_copy(st[:, 0:gw], ps[:, 0:gw])
                else:
                    nc.scalar.copy(st[:, 0:gw], ps[:, 0:gw])
                nc.sync.dma_start(
                    out=d_logits[c0:c0 + 128, v0:v0 + gw],
                    in_=st[:, 0:gw],
                )

            def proj_pieces(woutT, ci, g):
                """Two ~half-size filler pieces for one projection unit."""
                state = {}

                def piece_a():
                    ps = pp.tile([128, 1024], fp32, tag="proj", bufs=2)
                    state["ps"] = ps
                    _proj_half(woutT, ci, g, ps, (0, 1))

                def piece_b():
                    ps = state["ps"]
                    _proj_half(woutT, ci, g, ps, (2, 3))
                    _proj_evac(ci, g, ps)

                return [piece_a, piece_b]

            def proj_unit(woutT, ci, g):
                a, b = proj_pieces(woutT, ci, g)
                a()
                b()

            def step(t, ssc, sgt, fill_a=(), fill_b=()):
                rs = BL * (t - 1)

                # attention scores accumulate on top of the preA init
                for j in range(16):
                    for kk in range(4):
                        nc.tensor.matmul(
                            ssc[:, j, :],
                            wanhT[:, kk, 128 * j:128 * j + 128],
                            hidT[:, kk, rs:rs + BL],
                            start=False, stop=(kk == 3),
                            skip_group_check=True,
                        )
                # W_hh @ h into the gate bank; first chunk runs under the
                # exp latency, the rest after the denominator matmuls.
                def whh_chunk(jlo, jhi):
                    for j in range(jlo, jhi):
                        for kk in range(4):
                            nc.tensor.matmul(
                                sgt[:, j, :],
                                whhT[:, kk, 128 * j:128 * j + 128],
                                hidT[:, kk, rs:rs + BL],
                                start=False, stop=False,
                                skip_group_check=True,
                            )

                att = sc.tile([128, 16, BL], bf16, tag="att")
                nc.scalar.activation(att[:, :, :], ssc[:, 0:16, :], EXP)
                att2 = sc.tile([128, 16, BL], bf16, tag="att2")
                nc.vector.tensor_mul(att2[:, :, :], att[:, :, :], cnnT[:, :, :])

                whh_chunk(0, 10)

                # softmax denominator into ssc bank row (start=False onto the
                # zero fill; accumulates across the 16 j groups)
                ps_z = ssc[0:1, 16, :]
                for j in range(16):
                    nc.tensor.matmul(
                        ps_z,
                        onescol[:, 0:1],
                        att[:, j, :],
                        start=False, stop=(j == 15),
                        skip_group_check=True,
                    )

                whh_chunk(10, 16)

                # filler: keeps the PE busy while att2/recip land
                for f in fill_a:
                    f()
                rz = sc.tile([1, 4 * BL], fp32, tag="rz")
                nc.vector.reciprocal(rz[0:1, 0:BL], ps_z)
                for q in range(1, 4):
                    nc.vector.tensor_copy(rz[0:1, BL * q:BL * q + BL], rz[0:1, 0:BL])

                # attended-x2 (pre-normalization) into the gate bank's spare
                # groups; the 1/Z broadcast matmul is slotted mid-stream so
                # the PE reaches it just as the reciprocal lands.
                ps_rz = ssc[:, 20:24, :]
                for me in range(4):
                    for ka in range(16):
                        nc.tensor.matmul(
                            sgt[:, 16 + me, :],
                            wadT[:, ka, 128 * me:128 * me + 128],
                            att2[:, ka, :],
                            start=False, stop=(ka == 15),
                            skip_group_check=True,
                        )
                    if me == 2:
                        nc.tensor.matmul(
                            ps_rz, onesf[0:1, :], rz[0:1, :],
                            start=False, stop=True,
                            skip_group_check=True,
                        )
                rzbc = sc.tile([128, 4, BL], fp32, tag="rzbc")
                nc.vector.tensor_copy(rzbc[:, :, :], ps_rz)
                x2aT = sc.tile([128, 4, BL], bf16, tag="x2aT")
                nc.vector.tensor_mul(x2aT[:, :, :], sgt[:, 16:20, :], rzbc[:, :, :])

                # filler: keeps the PE busy while rzbc/x2aT land
                for f in fill_b:
                    f()

                for j in range(16):
                    for kk in range(4):
                        nc.tensor.matmul(
                            sgt[:, j, :],
                            wihT[:, kk, 128 * j:128 * j + 128],
                            x2aT[:, kk, :],
                            start=False, stop=(kk == 3),
                            skip_group_check=True,
                        )
                lstm_tail(sgt, t, False)

            # ---- prologue: step 0 + precompute half 1 -------------------
            with tc.tile_pool(name="pre", bufs=1) as ppre:
                wanxT = ppre.tile([128, 4, A], bf16)
                mxT = ppre.tile([128, 4, G4], bf16)
                xsT = ppre.tile([128, 4, RPAD], bf16)
                nc.sync.dma_start(out=xsT[:, :, :], in_=d_xsT[:, :, :])
                nc.sync.dma_start(out=wanxT[:, :, :], in_=d_wanxT[:, :])
                nc.sync.dma_start(out=mxT[:, :, :], in_=d_mxT[:, :])
                # step-1+ weights follow the precompute inputs on the queue
                nc.sync.dma_start(out=wanhT[:, :, :], in_=d_wanhT[:, :])
                nc.sync.dma_start(out=whhT[:, :, :], in_=d_whhT[:, :])
                nc.sync.dma_start(out=wadT[:, :, :], in_=d_wadT[:, :])
                nc.sync.dma_start(out=cnnT[:, :, :], in_=d_cnnT[:, :])

                # step 0: plain LSTM from zero state
                sg0 = pp.tile([128, 32, BL], fp32, tag="sgt", bufs=2)
                nc.tensor.matmul(
                    sg0[:, 0:16, :], onesI[:, :], preg0[:, :, :],
                    start=True, stop=False, skip_group_check=True,
                )
                nc.tensor.matmul(
                    sg0[:, 16:32, :], onesI[:, :], zrow[:, :, :],
                    start=False, stop=True, skip_group_check=True,
                )
                for j in range(16):
                    for kk in range(4):
                        nc.tensor.matmul(
                            sg0[:, j, :],
                            wihT[:, kk, 128 * j:128 * j + 128],
                            featT[:, kk, :],
                            start=False, stop=(kk == 3),
                            skip_group_check=True,
                        )
                lstm_tail(sg0, 0, True)

                # precompute half 1 (steps 1..16)
                for which in range(2):
                    for j in range(16):
                        precompute_chunk(ppre, wanxT, mxT, xsT, which, j, 0)

                # preloads for steps 1 and 2
                ssc1 = alloc_ssc(1)
                sgt1 = alloc_sgt(1)
                ssc2 = alloc_ssc(2)
                sgt2 = alloc_sgt(2)

                # remaining precompute (half 2) chunks, interleaved below
                pre2 = [(w, j) for w in range(2) for j in range(16)]
                pre2_sched = {}
                for i, u in enumerate(pre2):
                    pre2_sched.setdefault(1 + i * 7 // len(pre2), []).append(u)

                cur = (ssc1, sgt1)
                nxt = (ssc2, sgt2)
                for t in range(1, 8):
                    chunks = [
                        (lambda w=w, j=j:
                         precompute_chunk(ppre, wanxT, mxT, xsT, w, j, 1))
                        for (w, j) in pre2_sched.get(t, ())
                    ]
                    step(t, cur[0], cur[1],
                         fill_a=chunks[0:1], fill_b=chunks[1:2])
                    for f in chunks[2:]:
                        f()
                    nxt2 = (alloc_ssc(t + 2), alloc_sgt(t + 2))
                    cur, nxt = nxt, nxt2

            # wout loads after the pre pool frees its SBUF; per-group DMAs so
            # early projection units only wait on their own group.
            with tc.tile_pool(name="wout", bufs=1) as pwo:
                woutT = pwo.tile([128, 4, V], bf16)
                for g in range(NVG):
                    v0 = VGW * g
                    gw = min(VGW, V - v0)
                    nc.sync.dma_start(
                        out=woutT[:, :, v0:v0 + gw],
                        in_=d_woutT[:, :, v0:v0 + gw],
                    )

                # chunk ci (hidden cols 128*ci..) is final after step 8*ci+7;
                # spread its 10 vocab groups over steps 8*ci+8 .. 8*ci+15.
                proj_sched = {}
                for ci in range(NCH - 1):
                    for s in range(8):
                        t0 = 8 * ci + 8 + s
                        units = range(NVG * s // 8, NVG * (s + 1) // 8)
                        proj_sched.setdefault(t0, []).extend((ci, g) for g in units)

                for t in range(8, NT):
                    pieces = []
                    for (ci, g) in proj_sched.get(t, ()):
                        pieces.extend(proj_pieces(woutT, ci, g))
                    step(t, cur[0], cur[1],
                         fill_a=pieces[0:1], fill_b=pieces[1:2])
                    for f in pieces[2:]:
                        f()
                    if t + 2 < NT:
                        nxt2 = (alloc_ssc(t + 2), alloc_sgt(t + 2))
                    else:
                        nxt2 = (None, None)
                    cur, nxt = nxt, nxt2

                for g in range(NVG):
                    proj_unit(woutT, NCH - 1, g)

    # post-pass: walrus in this container allows only 1 sem wait per
    # instruction; move extras onto same-engine NoOps inserted just before.
    nid = 0
    for f in nc.m.functions:
        for bb in f.blocks:
            insts = bb.instructions
            i = 0
            while i < len(insts):
                ins = insts[i]
                si = ins.sync_info
                if si is not None and len(si.on_wait) > 1:
                    waits = list(si.on_wait)
                    si.on_wait = waits[-1:]
                    for w in waits[:-1]:
                        nid += 1
                        nop = mybir.InstNoOp(
                            name=f"WS-{nid}",
                            sync_info=mybir.SyncInfo(on_wait=[w], on_update=[]),
                            bass_nofuse=True,
                            engine=ins.engine,
                        )
                        insts.insert(i, nop)
                        i += 1
                i += 1
    return nc


def _prep_inputs(inputs):
    f32 = np.float32
    features = np.asarray(inputs["features"], f32)
    cnn = np.asarray(inputs["cnn_features"], f32)
    captions = np.asarray(inputs["captions"])
    emb = np.asarray(inputs["embed_table"], f32)
    W_ih = np.asarray(inputs["W_ih"], f32)
    W_hh = np.asarray(inputs["W_hh"], f32)
    b_ih = np.asarray(inputs["b_ih"], f32)
    b_hh = np.asarray(inputs["b_hh"], f32)
    W_attn = np.asarray(inputs["W_attn"], f32)
    b_attn = np.asarray(inputs["b_attn"], f32)
    W_attd = np.asarray(inputs["W_attd"], f32)
    b_attd = np.asarray(inputs["b_attd"], f32)
    W_out = np.asarray(inputs["W_out"], f32)

    s = np.ones((G4, 1), f32)
    s[0:H] = 0.5
    s[H:2 * H] = 0.5
    s[3 * H:4 * H] = 0.5
    Mx = W_ih @ W_attd[:, :E]
    bias_g = (b_ih + b_hh + W_ih @ b_attd) * s[:, 0]
    bias_g0 = (b_ih + b_hh) * s[:, 0]
    preg0 = np.repeat(bias_g0[:, None], BL, axis=1)       # [G4, BL]

    common = {
        "wanhT": _f8(_fmajor(np.asarray(W_attn[:, E:].T, np.float32))),
        "wanxT": _fmajor(_bf(W_attn[:, :E].T)),
        "wadT": _f8(_fmajor(np.asarray(W_attd[:, E:].T, np.float32))),
        "mxT": _fmajor(_bf((Mx * s).T)),
        "wihT": _fmajor(_bf((W_ih * s).T)),
        "whhT": _fmajor(_bf((W_hh * s).T)),
        "woutT": _fmajor(_bf(W_out.T)).reshape(128, 4, V),
        "battn": _bf(b_attn[None, :]),
        "bg": _bf(bias_g[None, :]),
        "preg0": _fmajor(_bf(preg0)),
        "onesrow": _bf(np.ones((1, RPAD), f32)),
        "onesf": np.ones((1, 128), f32),
        "onescol": _bf(np.ones((128, 1), f32)),
        "onesI": _bf(np.eye(128, dtype=f32)),
        "zrow": _bf(np.zeros((128, 16 * BL), f32)),
    }
    in_maps = []
    for k in range(NCORES):
        bsl = slice(BL * k, BL * k + BL)
        toks = captions[bsl].astype(np.int64).T.reshape(-1)   # r=(t-1)*16+b
        xs = np.zeros((RPAD, E), np.float32)
        xs[:R] = emb[toks]
        in_maps.append({
            **common,
            "xsT": _fmajor(_bf(xs.T)).reshape(128, 4, RPAD),
            "featT": _fmajor(_bf(features[bsl].T)),
            "cnnT": _fmajor(_bf(cnn[bsl].T)),
        })
    return in_maps


def kernel(**inputs):
    from concourse.bass_utils import run_bass_kernel_spmd

    if "nc" not in _BUILT:
        _BUILT["nc"] = _build_program()
    nc = _BUILT["nc"]
    in_maps = _prep_inputs(inputs)
    res = run_bass_kernel_spmd(nc, in_maps, list(range(NCORES)))

    b_out = np.asarray(inputs["b_out"], np.float32)
    out = np.empty((NT * B, V), np.float32)
    o3 = out.reshape(NT, B, V)
    for k in range(NCORES):
        lt = res.results[k]["logits"]                        # [512, V] bf16
        o3[:, BL * k:BL * k + BL, :] = np.asarray(lt, np.float32).reshape(NT, BL, V)
    out += b_out[None, :]
    return out
